# revision 1
# baseline (speedup 1.0000x reference)
"""Mamba-1 style selective scan on 8 Trainium2 NeuronCores.

Sharding: core c -> (batch b = c//2, D-half h = c%2).  Each core receives
x[b] with its local 512 channels permuted to the front (weights permuted to
match), computes y^T[512, T] for its channels, host reassembles.

On-chip layout: partitions = (d_sub in 0..7) x (n in 0..15) "groups" of
8 channels x 16 states; free dim = time (chunks of TC).  The recurrence
   s_t = A_bar*s + (A_bar-1)/A * x_t * B_t,   y_t = sum_n s_t*C_t + D*x
runs as: PE replicates dt/x across state-partitions and contracts y over n;
ACT evaluates exp/softplus; DVE builds the scan inputs and runs the
hardware linear scan (tensor_tensor_scan) along the time axis.
"""

import sys

import numpy as np

sys.path.insert(0, "/opt/trn_rl_repo")

import ml_dtypes

import concourse.bacc as bacc
import concourse.mybir as mybir
import concourse.tile as tile
from concourse.bass_utils import run_bass_kernel_spmd

B, T, D, N, R = 4, 4096, 1024, 16, 64
NCORES = 8
DH = D // 2            # channels per core
TC = 1024              # time chunk for DVE ops
PH = 512               # PSUM half (one bank of f32)
NCH = T // TC
NDT = DH // 128        # 128-channel tiles per core (4)
NG = DH * N // 128     # (d,n) partition groups per core (64)
GPD = NG // NDT        # groups per d-tile (16)
F32 = mybir.dt.float32
BF16 = mybir.dt.bfloat16
SC_BF16 = True
AL = mybir.AluOpType
AF = mybir.ActivationFunctionType

_CACHE = {}


def _patch_act_tables():
    """Make the act-table pass pick natural_log_exp_and_others for Exp+Ln
    (same table indices; strip Exp/Ln from the single-func tables so the
    combined one is the only candidate -> no per-chunk LUT reload ping-pong)."""
    import concourse.bacc as _bacc
    from concourse.hw_specs import get_activation_tables as _orig

    def patched(arch):
        t = _orig(arch)
        exp = mybir.ActivationFunctionType.Exp
        ln = mybir.ActivationFunctionType.Ln
        for name, fns in t.items():
            if name != "natural_log_exp_and_others":
                fns.discard(exp)
                fns.discard(ln)
        return t

    _bacc.get_activation_tables = patched


def _build_program():
    _patch_act_tables()
    nc = bacc.Bacc(
        "TRN2",
        target_bir_lowering=False,
        debug=False,
        num_devices=NCORES,
    )

    x_d = nc.dram_tensor("x", [T, D], F32, kind="ExternalInput")
    st_d = nc.dram_tensor("state_r", [128, NG], F32, kind="ExternalInput")
    a_d = nc.dram_tensor("a_rep", [128, NG], F32, kind="ExternalInput")
    ia_d = nc.dram_tensor("inva_rep", [128, NG], F32, kind="ExternalInput")
    wb_d = nc.dram_tensor("wb_t", [D, N], F32, kind="ExternalInput")
    wc_d = nc.dram_tensor("wc_t", [D, N], F32, kind="ExternalInput")
    w1_d = nc.dram_tensor("wdt1_t", [D, R], F32, kind="ExternalInput")
    w2_d = nc.dram_tensor("wdt2_t", [R, DH], F32, kind="ExternalInput")
    bd_d = nc.dram_tensor("bdt2", [128, NDT], F32, kind="ExternalInput")
    dsk_d = nc.dram_tensor("dskdiag", [128, NDT * 128], F32,
                           kind="ExternalInput")
    sel8_d = nc.dram_tensor("sel128", [128, GPD * 128], F32, kind="ExternalInput")
    seln_d = nc.dram_tensor("seln", [N, 128], F32, kind="ExternalInput")
    nsum_d = nc.dram_tensor("nsum128", [128, GPD * 128],
                            BF16 if SC_BF16 else F32, kind="ExternalInput")
    ident_d = nc.dram_tensor("ident", [128, 128], F32, kind="ExternalInput")
    y_d = nc.dram_tensor("yT", [DH, T], F32, kind="ExternalOutput")

    with tile.TileContext(nc) as tc:
        _body(tc, x_d, st_d, a_d, ia_d, wb_d, wc_d, w1_d, w2_d, bd_d,
              dsk_d, sel8_d, seln_d, nsum_d, ident_d, y_d)

    nc.compile()
    return nc


def _body(tc, x_d, st_d, a_d, ia_d, wb_d, wc_d, w1_d, w2_d, bd_d,
          dsk_d, sel8_d, seln_d, nsum_d, ident_d, y_d):
    nc = tc.nc
    KD = D // 128  # k-tiles over full D for the projections (8)

    with (
        tc.tile_pool(name="const", bufs=1) as const,
        tc.tile_pool(name="xload", bufs=3) as xload,
        tc.tile_pool(name="xt", bufs=2 * KD + 1) as xtp,
        tc.tile_pool(name="proj", bufs=2) as projp,
        tc.tile_pool(name="dtp", bufs=NDT) as dtp,
        tc.tile_pool(name="dtep", bufs=1) as dtep,
        tc.tile_pool(name="rep", bufs=2) as repp,
        tc.tile_pool(name="work", bufs=2) as workp,
        tc.tile_pool(name="scan", bufs=3) as scanp,
        tc.tile_pool(name="yout", bufs=2) as youtp,
        tc.tile_pool(name="ps_t", bufs=1, space="PSUM") as ps_t,
        tc.tile_pool(name="ps_proj", bufs=1, space="PSUM") as ps_proj,
        tc.tile_pool(name="ps_rep", bufs=4, space="PSUM") as ps_rep,
        tc.tile_pool(name="ps_y", bufs=2, space="PSUM") as ps_y,
    ):
        # ---- constants / small state ----
        ident = const.tile([128, 128], F32)
        nc.gpsimd.dma_start(ident, ident_d[:, :])
        sel128 = const.tile([128, GPD, 128], F32)
        nc.gpsimd.dma_start(sel128, sel8_d.ap().rearrange("k (s p) -> k s p", s=GPD))
        seln = const.tile([N, 128], F32)
        nc.gpsimd.dma_start(seln, seln_d[:, :])
        nsum128 = const.tile([128, GPD, 128], BF16 if SC_BF16 else F32)
        nc.gpsimd.dma_start(nsum128, nsum_d.ap().rearrange("k (s p) -> k s p", s=GPD))
        arep = const.tile([128, NG], F32)
        nc.gpsimd.dma_start(arep, a_d[:, :])
        iarep = const.tile([128, NG], F32)
        nc.gpsimd.dma_start(iarep, ia_d[:, :])
        bdt2 = const.tile([128, NDT], F32)
        nc.gpsimd.dma_start(bdt2, bd_d[:, :])
        dskdiag = const.tile([128, NDT, 128], F32)
        nc.gpsimd.dma_start(
            dskdiag, dsk_d.ap().rearrange("k (d p) -> k d p", d=NDT))
        carry = const.tile([128, NG], F32)
        nc.gpsimd.dma_start(carry, st_d[:, :])

        wb = const.tile([128, KD, N], F32)
        nc.gpsimd.dma_start(wb, wb_d.ap().rearrange("(k p) n -> p k n", p=128))
        wc = const.tile([128, KD, N], F32)
        nc.gpsimd.dma_start(wc, wc_d.ap().rearrange("(k p) n -> p k n", p=128))
        w1 = const.tile([128, KD, R], F32)
        nc.gpsimd.dma_start(w1, w1_d.ap().rearrange("(k p) r -> p k r", p=128))
        w2 = const.tile([R, DH], F32)
        nc.gpsimd.dma_start(w2, w2_d[:, :])

        def frontend(ch):
            t0 = ch * TC
            # ---- load x chunk and transpose: xt[db] = x[t0:t0+TC, :].T ----
            xt = [xtp.tile([128, TC], F32, tag="xt", name=f"xt{db}")
                  for db in range(KD)]
            for tp2 in range(TC // 256):
                xl = []
                for j in range(2):
                    tt = 2 * tp2 + j
                    xld = xload.tile([128, D], F32, tag="xld", name=f"xld{j}")
                    nc.sync.dma_start(
                        xld, x_d[t0 + tt * 128: t0 + (tt + 1) * 128, :])
                    xl.append(xld)
                for db in range(KD):
                    pt = ps_t.tile([128, 256], F32, tag="tp")
                    for j in range(2):
                        nc.tensor.transpose(
                            pt[:, j * 128:(j + 1) * 128],
                            xl[j][:, db * 128:(db + 1) * 128], ident
                        )
                    nc.scalar.copy(
                        xt[db][:, tp2 * 256:(tp2 + 1) * 256], pt)

            # ---- projections over full D: xr[r,t], Bt[n,t], Ct[n,t] ----
            xr = projp.tile([R, TC], F32)
            bt = projp.tile([N, TC], F32)
            ct = projp.tile([N, TC], F32)
            brep = repp.tile([128, TC], F32)
            crep = repp.tile([128, TC], BF16 if SC_BF16 else F32)
            for hf in range(TC // PH):
                hs = slice(hf * PH, (hf + 1) * PH)
                pxr = ps_proj.tile([R, PH], F32, tag="proj")
                for k in range(KD):
                    nc.tensor.matmul(pxr, w1[:, k, :], xt[k][:, hs],
                                     start=(k == 0), stop=(k == KD - 1))
                nc.scalar.copy(xr[:, hs], pxr)
                pb = ps_proj.tile([N, PH], F32, tag="proj")
                for k in range(KD):
                    nc.tensor.matmul(pb, wb[:, k, :], xt[k][:, hs],
                                     start=(k == 0), stop=(k == KD - 1))
                nc.scalar.copy(bt[:, hs], pb)
                pc = ps_proj.tile([N, PH], F32, tag="proj")
                for k in range(KD):
                    nc.tensor.matmul(pc, wc[:, k, :], xt[k][:, hs],
                                     start=(k == 0), stop=(k == KD - 1))
                nc.scalar.copy(ct[:, hs], pc)
                prb = ps_rep.tile([128, PH], F32, tag="rep")
                nc.tensor.matmul(prb, seln, bt[:, hs], start=True, stop=True)
                nc.scalar.copy(brep[:, hs], prb)
                prc = ps_rep.tile([128, PH], F32, tag="rep")
                nc.tensor.matmul(prc, seln, ct[:, hs], start=True, stop=True)
                nc.scalar.copy(crep[:, hs], prc)

            # ---- dt per d-tile: softplus(W2 @ xr + b) ----
            dts = []
            for dtl in range(NDT):
                dtt = dtp.tile([128, TC], F32, tag="dtt", name=f"dtt{dtl}")
                for hf in range(TC // PH):
                    hs = slice(hf * PH, (hf + 1) * PH)
                    pdt = ps_proj.tile([128, PH], F32, tag="proj")
                    nc.tensor.matmul(pdt, w2[:, dtl * 128:(dtl + 1) * 128],
                                     xr[:, hs], start=True, stop=True)
                    nc.scalar.activation(dtt[:, hs], pdt, AF.Exp,
                                         bias=bdt2[:, dtl:dtl + 1], scale=1.0)
                nc.scalar.activation(dtt, dtt, AF.Ln, bias=1.0, scale=1.0)
                dts.append(dtt)

            return xt, brep, crep, dts

        for ch in range(NCH):
            t0 = ch * TC
            xt, brep, crep, dts = frontend(ch)
            # ---- per (d-tile, group): the recurrence ----
            for dtl in range(NDT):
                pys = [ps_y.tile([128, PH], F32, tag="y", name=f"py{hf}")
                       for hf in range(TC // PH)]
                for q in range(GPD // 2):
                  for sub in (q, q + GPD // 2):
                    g = dtl * GPD + sub
                    rg = 0 if sub < GPD // 2 else 1
                    rsl = slice(rg * 64, rg * 64 + 64)

                    at = workp.tile([128, TC], F32)
                    gt = workp.tile([128, TC], F32)
                    for hf in range(TC // PH):
                        hs = slice(hf * PH, (hf + 1) * PH)
                        pdr = ps_rep.tile([128, PH], F32, tag="rep")
                        nc.tensor.matmul(pdr, sel128[rsl, sub, :],
                                         dts[dtl][rsl, hs],
                                         start=True, stop=True,
                                         tile_position=(rg * 64, 0))
                        nc.scalar.activation(at[:, hs], pdr, AF.Exp,
                                             scale=arep[:, g:g + 1])
                        pxrep = ps_rep.tile([128, PH], F32, tag="rep")
                        nc.tensor.matmul(pxrep, sel128[rsl, sub, :],
                                         xt[dtl][rsl, hs],
                                         start=True, stop=True,
                                         tile_position=(rg * 64, 0))
                        nc.vector.scalar_tensor_tensor(
                            gt[:, hs], pxrep, iarep[:, g:g + 1],
                            brep[:, hs], op0=AL.mult, op1=AL.mult)

                    ut = workp.tile([128, TC], F32)
                    nc.vector.scalar_tensor_tensor(
                        ut, at, -1.0, gt, op0=AL.add, op1=AL.mult)

                    st = scanp.tile([128, TC], BF16 if SC_BF16 else F32)
                    nc.vector.tensor_tensor_scan(
                        st, at, ut, carry[:, g:g + 1],
                        op0=AL.mult, op1=AL.add)
                    nc.scalar.copy(carry[:, g:g + 1], st[:, TC - 1:TC])

                    sct = scanp.tile([128, TC], BF16 if SC_BF16 else F32)
                    nc.vector.tensor_tensor(sct, st, crep, AL.mult)
                    for hf in range(TC // PH):
                        hs = slice(hf * PH, (hf + 1) * PH)
                        nc.tensor.matmul(pys[hf][rsl, :],
                                         nsum128[:, sub, rsl],
                                         sct[:, hs],
                                         start=(q == 0), stop=False,
                                         tile_position=(0, rg * 64))
                for hf in range(TC // PH):
                    hs = slice(hf * PH, (hf + 1) * PH)
                    nc.tensor.matmul(pys[hf], dskdiag[:, dtl, :],
                                     xt[dtl][:, hs],
                                     start=False, stop=True)

                yo = youtp.tile([128, TC], F32, tag="yo", name="yo")
                for hf in range(TC // PH):
                    nc.scalar.copy(yo[:, hf * PH:(hf + 1) * PH], pys[hf])
                nc.sync.dma_start(
                    y_d[dtl * 128:(dtl + 1) * 128, t0:t0 + TC], yo)


def _selectors():
    p = np.arange(128)
    k = np.arange(128)
    # sel128[s][k, p] = 1 iff k == s*8 + p//16  (replicate 8 rows over n)
    sel = np.stack([(k[:, None] == s * 8 + p[None, :] // 16)
                    for s in range(GPD)]).astype(np.float32)
    # nsum128[s][k, m] = 1 iff m == s*8 + k//16  (contract n into row block s)
    nsm = np.stack([(p[None, :] == s * 8 + k[:, None] // 16)
                    for s in range(GPD)]).astype(np.float32)
    # SBUF layout [k, s, p] flattened to [128, GPD*128]
    sel128 = np.ascontiguousarray(
        np.transpose(sel, (1, 0, 2)).reshape(128, GPD * 128))
    nsum128 = np.ascontiguousarray(
        np.transpose(nsm, (1, 0, 2)).reshape(128, GPD * 128))
    seln = (p[None, :] % 16 == np.arange(N)[:, None]).astype(np.float32)
    ident = np.eye(128, dtype=np.float32)
    return sel128, seln, nsum128, ident


def _dskdiag(dsk):
    """[512] -> [128, NDT*128]: per d-tile diagonal matrices, laid out
    [k, (d, p)] so sbuf tile [128, NDT, 128] slices to diag(dsk[dtl])."""
    out = np.zeros((128, NDT, 128), np.float32)
    for d in range(NDT):
        out[np.arange(128), d, np.arange(128)] = dsk[d * 128:(d + 1) * 128]
    return np.ascontiguousarray(out.reshape(128, NDT * 128))


def _rearr(m):
    """[512, 16] (d, n) -> [128, 64]: column g holds group g, row p=(d_sub*16+n)."""
    return np.ascontiguousarray(
        m.reshape(NG, 8, N).reshape(NG, 128).T)


def kernel(x, state, log_A, W_B, W_C, W_dt1, W_dt2, b_dt2, D_skip):
    if "nc" not in _CACHE:
        _CACHE["nc"] = _build_program()
    nc = _CACHE["nc"]

    x = np.asarray(x, np.float32)
    state = np.asarray(state, np.float32)
    A = (-np.exp(np.asarray(log_A, np.float32))).astype(np.float32)
    invA = (np.float32(1.0) / (A + np.float32(1e-8))).astype(np.float32)
    W_B = np.asarray(W_B, np.float32)
    W_C = np.asarray(W_C, np.float32)
    W_dt1 = np.asarray(W_dt1, np.float32)
    W_dt2 = np.asarray(W_dt2, np.float32)
    b_dt2 = np.asarray(b_dt2, np.float32)
    D_skip = np.asarray(D_skip, np.float32)

    sel128, seln, nsum128, ident = _selectors()

    in_maps = []
    for c in range(NCORES):
        b, h = c // 2, c % 2
        loc = slice(h * DH, (h + 1) * DH)
        oth = slice((1 - h) * DH, (2 - h) * DH)
        perm = np.r_[np.arange(h * DH, (h + 1) * DH),
                     np.arange((1 - h) * DH, (2 - h) * DH)]
        in_maps.append({
            "x": np.ascontiguousarray(x[b][:, perm]),
            "state_r": _rearr(state[b, loc]),
            "a_rep": _rearr(A[loc]),
            "inva_rep": _rearr(invA[loc]),
            "wb_t": np.ascontiguousarray(W_B.T[perm]),
            "wc_t": np.ascontiguousarray(W_C.T[perm]),
            "wdt1_t": np.ascontiguousarray(W_dt1.T[perm]),
            "wdt2_t": np.ascontiguousarray(W_dt2[loc].T),
            "bdt2": np.ascontiguousarray(b_dt2[loc].reshape(NDT, 128).T),
            "dskdiag": _dskdiag(D_skip[loc]),
            "sel128": sel128,
            "seln": seln,
            "nsum128": (nsum128.astype(ml_dtypes.bfloat16)
                        if SC_BF16 else nsum128),
            "ident": ident,
        })

    _CACHE["last_in_maps"] = in_maps
    res = run_bass_kernel_spmd(nc, in_maps, core_ids=list(range(NCORES)))

    y = np.empty((B, T, D), np.float32)
    for c in range(NCORES):
        b, h = c // 2, c % 2
        y[b][:, h * DH:(h + 1) * DH] = res.results[c]["yT"].T
    return y



# revision 8
# speedup vs baseline: 1.5615x; 1.5615x over previous
"""Mamba-1 selective scan on 8 Trainium2 NeuronCores — n-in-free-dim design.

Sharding: core c -> (batch b = c//2, D-half h = c%2): each core owns 512
channels of one batch for the recurrence; projections need the full D=1024.

Math (exact ZOH, rescaled state):
  G = A + 1e-8,  shat := G * s
  a_t = exp(dt_t * A)                           (per d,n,t)
  shat_t = a_t shat_{t-1} + (a_t - 1) ghat_t,   ghat = x * B
  w := shat + ghat  ->  w_t = (delta_t + w_{t-1}) * a_t,
       delta_t = ghat_t - ghat_{t-1}            (hw tensor_tensor_scan)
  y_t[d] = sum_n (1/G)[d,n] (w - ghat) C[n,t] + Dskip[d] x[d,t]
         = [sum_n diag(1/G_n) @ (w_n * crep_n)]  - x*q + Dskip*x
    q[d,t] = sum_n (1/G)[d,n] B[n,t] C[n,t]     (PE matmul of bc = B*C)

On-chip layout: partitions = 128 channels (4 d-tiles per core); n and time
in the free dim: per (n, dtile, chunk) tiles [128, TC].  All elementwise in
fp16 for the DVE 2x mode; scans are 1x; a is fp16 (exp computed f32 on ACT).
xt/pall columns are time-shifted by +1 (col 0 = time -1 = zeros) so the
delta at chunk boundaries needs no carry.
"""

import sys

import numpy as np

sys.path.insert(0, "/opt/trn_rl_repo")

import concourse.bacc as bacc
import concourse.mybir as mybir
import concourse.tile as tile
from concourse.bass_utils import run_bass_kernel_spmd

B, T, D, N, R = 4, 4096, 1024, 16, 64
NCORES = 8
DH = D // 2            # channels per core
NDT = DH // 128        # d-tiles per core (4)
KD = D // 128          # k-tiles over full D for projections (8)
TC = 1024              # time chunk
NCH = T // TC
PH = 512               # psum piece (one bank of f32)
NPC = TC // PH         # psum pieces per chunk (2)
F32 = mybir.dt.float32
FP16 = mybir.dt.float16
AL = mybir.AluOpType
AF = mybir.ActivationFunctionType

# engine assignment for the sct multiply: give Pool every POOL_SCT-th tile
POOL_SCT = 2

_CACHE = {}


def _patch_act_tables():
    """Route Exp+Ln to natural_log_exp_and_others so the softplus (Exp,Ln)
    and the main-loop Exp never force activation-table reloads."""
    import concourse.bacc as _bacc
    from concourse.hw_specs import get_activation_tables as _orig

    def patched(arch):
        t = _orig(arch)
        exp = mybir.ActivationFunctionType.Exp
        ln = mybir.ActivationFunctionType.Ln
        for name, fns in t.items():
            if name != "natural_log_exp_and_others":
                fns.discard(exp)
                fns.discard(ln)
        return t

    _bacc.get_activation_tables = patched


def _build_program():
    _patch_act_tables()
    nc = bacc.Bacc(
        "TRN2",
        target_bir_lowering=False,
        debug=False,
        num_devices=NCORES,
    )

    x_d = nc.dram_tensor("x16", [T, D], FP16, kind="ExternalInput")
    wall_d = nc.dram_tensor("wall", [128, KD * 112], FP16, kind="ExternalInput")
    w2_d = nc.dram_tensor("w2r", [64, NDT * 128], FP16, kind="ExternalInput")
    bd_d = nc.dram_tensor("bdt2", [128, NDT], F32, kind="ExternalInput")
    ac_d = nc.dram_tensor("acols", [128, NDT * N], F32, kind="ExternalInput")
    dgw_d = nc.dram_tensor("dgw", [128, NDT * N * 128], FP16,
                           kind="ExternalInput")
    dsk_d = nc.dram_tensor("dskw", [128, NDT * 128], FP16,
                           kind="ExternalInput")
    qw_d = nc.dram_tensor("qw", [16, NDT * 128], FP16, kind="ExternalInput")
    sel_d = nc.dram_tensor("selbc", [128, 2 * N * 128], FP16, kind="ExternalInput")
    nid_d = nc.dram_tensor("nident", [128, 128], FP16, kind="ExternalInput")
    id16_d = nc.dram_tensor("ident16", [128, 128], FP16, kind="ExternalInput")
    w0_d = nc.dram_tensor("w0init", [128, NDT * N], F32, kind="ExternalInput")
    y_d = nc.dram_tensor("yT", [DH, T], FP16, kind="ExternalOutput")

    with tile.TileContext(nc) as tc:
        _body(tc, x_d, wall_d, w2_d, bd_d, ac_d, dgw_d, dsk_d, qw_d, sel_d,
              nid_d, id16_d, w0_d, y_d)

    nc.compile()
    return nc


def _body(tc, x_d, wall_d, w2_d, bd_d, ac_d, dgw_d, dsk_d, qw_d, sel_d,
          nid_d, id16_d, w0_d, y_d):
    nc = tc.nc

    with (
        tc.tile_pool(name="const", bufs=1) as const,
        tc.tile_pool(name="xload", bufs=2) as xload,
        tc.tile_pool(name="xtmp", bufs=3) as xtmpp,
        tc.tile_pool(name="bcache", bufs=1) as bcache,
        tc.tile_pool(name="dtp", bufs=1) as dtp,
        tc.tile_pool(name="work", bufs=2) as workp,
        tc.tile_pool(name="gwork", bufs=2) as gworkp,
        tc.tile_pool(name="scan", bufs=2) as scanp,
        tc.tile_pool(name="sctp", bufs=2) as sctp,
        tc.tile_pool(name="yout", bufs=2) as youtp,
        tc.tile_pool(name="psA", bufs=2, space="PSUM") as psA,
        tc.tile_pool(name="psB", bufs=2, space="PSUM") as psB,
        tc.tile_pool(name="psY", bufs=4, space="PSUM") as psY,
    ):
        # ---- constants ----
        wall = const.tile([128, KD, 112], FP16)
        nc.gpsimd.dma_start(wall, wall_d.ap().rearrange("p (k m) -> p k m",
                                                        k=KD))
        w2r = const.tile([64, NDT, 128], FP16)
        nc.gpsimd.dma_start(w2r, w2_d.ap().rearrange("p (d m) -> p d m",
                                                     d=NDT))
        bdt2 = const.tile([128, NDT], F32)
        nc.gpsimd.dma_start(bdt2, bd_d[:, :])
        acols = const.tile([128, NDT * N], F32)
        nc.gpsimd.dma_start(acols, ac_d[:, :])
        dgw = const.tile([128, NDT * N, 128], FP16)
        nc.gpsimd.dma_start(dgw, dgw_d.ap().rearrange("p (g m) -> p g m",
                                                      g=NDT * N))
        dskw = const.tile([128, NDT, 128], FP16)
        nc.gpsimd.dma_start(dskw, dsk_d.ap().rearrange("p (d m) -> p d m",
                                                       d=NDT))
        qw = const.tile([16, NDT, 128], FP16)
        nc.gpsimd.dma_start(qw, qw_d.ap().rearrange("p (d m) -> p d m",
                                                    d=NDT))
        selbc = const.tile([128, 2 * N, 128], FP16)
        nc.gpsimd.dma_start(selbc, sel_d.ap().rearrange("p (n m) -> p n m",
                                                        n=2 * N))
        nident = const.tile([128, 128], FP16)
        nc.gpsimd.dma_start(nident, nid_d[:, :])
        ident16 = const.tile([128, 128], FP16)
        nc.gpsimd.dma_start(ident16, id16_d[:, :])
        wc = const.tile([128, NDT * N], F32)
        nc.gpsimd.dma_start(wc, w0_d[:, :])

        # ---- transposes + projections (time-shifted by +1 col) ----
        # xt[:, dtl, 1+t] = x[t, dtl*128 + p]; col 0 = 0
        xt = const.tile([128, NDT, T + 1], FP16)
        nc.vector.memset(xt[:, :, 0:1], 0.0)
        # pall[0:64]=xr, [64:80]=Bt, [96:112]=Ct (80:96 pad); col 0 = 0
        pall = const.tile([112, T + 1], FP16)
        nc.vector.memset(pall[:, 0:1], 0.0)

        for tp in range(T // PH):
            xls = []
            for j in range(4):
                xld = xload.tile([128, D], FP16, tag=f"xld{j}",
                                 name=f"xld{j}")
                nc.sync.dma_start(
                    xld, x_d[tp * PH + j * 128: tp * PH + (j + 1) * 128, :])
                xls.append(xld)
            ktiles = []
            for k in range(KD):
                ptr = psA.tile([128, PH], FP16, tag="psA")
                for j in range(4):
                    nc.tensor.transpose(
                        ptr[:, j * 128:(j + 1) * 128],
                        xls[j][:, k * 128:(k + 1) * 128], ident16)
                if k < NDT:
                    dst = xt[:, k, 1 + tp * PH: 1 + (tp + 1) * PH]
                    nc.scalar.copy(dst, ptr)
                    ktiles.append(xt[:, k, 1 + tp * PH: 1 + (tp + 1) * PH])
                else:
                    xtm = xtmpp.tile([128, PH], FP16, tag="xtm",
                                     name=f"xtm{k}")
                    nc.scalar.copy(xtm, ptr)
                    ktiles.append(xtm)
            pp = psB.tile([112, PH], F32, tag="psB")
            for k in range(KD):
                nc.tensor.matmul(pp, wall[:, k, :], ktiles[k],
                                 start=(k == 0), stop=(k == KD - 1))
            nc.scalar.copy(pall[:, 1 + tp * PH: 1 + (tp + 1) * PH], pp)

        itercnt = 0
        for ch in range(NCH):
            t0 = ch * TC

            # ---- dt for this chunk: softplus(w2 @ xr + b) ----
            dts = dtp.tile([128, NDT, TC], FP16, tag="dts", name="dts")
            for dtl in range(NDT):
                for hf in range(NPC):
                    sl = slice(1 + t0 + hf * PH, 1 + t0 + (hf + 1) * PH)
                    pdt = psB.tile([128, PH], F32, tag="psB")
                    nc.tensor.matmul(pdt, w2r[:, dtl, :], pall[0:64, sl],
                                     start=True, stop=True)
                    dsl = dts[:, dtl, hf * PH:(hf + 1) * PH]
                    nc.scalar.activation(dsl, pdt, AF.Exp,
                                         bias=bdt2[:, dtl:dtl + 1], scale=1.0)
                nc.scalar.activation(dts[:, dtl, :], dts[:, dtl, :],
                                     AF.Ln, bias=1.0, scale=1.0)

            # ---- bc = B*C for the q correction (copies realign base) ----
            btc = workp.tile([16, TC], FP16, tag="btc", name="btc")
            nc.scalar.copy(btc, pall[64:80, 1 + t0: 1 + t0 + TC])
            ctc = workp.tile([16, TC], FP16, tag="ctc", name="ctc")
            nc.scalar.copy(ctc, pall[96:112, 1 + t0: 1 + t0 + TC])
            bc = workp.tile([16, TC], FP16, tag="bc", name="bc")
            nc.vector.tensor_tensor(bc, btc, ctc, AL.mult)

            # ---- broadcast B and C rows to 128 partitions ----
            # brep1[n] covers times t0-1 .. t0+TC-1  (pall cols t0..t0+TC)
            # crep[n]  covers times t0   .. t0+TC-1  (pall cols 1+t0..)
            breps, creps = [], []
            for n in range(N):
                br = bcache.tile([128, TC + 1], FP16, tag=f"br{n}",
                                 name=f"br{n}")
                for hf in range(NPC):
                    pb = psA.tile([128, PH], F32, tag="psA")
                    nc.tensor.matmul(
                        pb, selbc[64:112, n, :],
                        pall[64:112, t0 + hf * PH: t0 + (hf + 1) * PH],
                        start=True, stop=True, tile_position=(64, 0))
                    nc.scalar.copy(br[:, hf * PH:(hf + 1) * PH], pb)
                pbl = psA.tile([128, PH], F32, tag="psA")
                nc.tensor.matmul(pbl[:, 0:1], selbc[64:112, n, :],
                                 pall[64:112, t0 + TC: t0 + TC + 1],
                                 start=True, stop=True, tile_position=(64, 0))
                nc.scalar.copy(br[:, TC: TC + 1], pbl[:, 0:1])
                breps.append(br)

                cr = bcache.tile([128, TC], FP16, tag=f"cr{n}",
                                 name=f"cr{n}")
                for hf in range(NPC):
                    pc = psA.tile([128, PH], F32, tag="psA")
                    nc.tensor.matmul(
                        pc, selbc[64:112, N + n, :],
                        pall[64:112, 1 + t0 + hf * PH: 1 + t0 + (hf + 1) * PH],
                        start=True, stop=True, tile_position=(64, 0))
                    nc.scalar.copy(cr[:, hf * PH:(hf + 1) * PH], pc)
                creps.append(cr)

            # ---- recurrence per (dtile, n) ----
            for dtl in range(NDT):
                pys = [psY.tile([128, PH], F32, tag="psY", name=f"py{hf}")
                       for hf in range(NPC)]
                for n in range(N):
                    g = dtl * N + n
                    at = workp.tile([128, TC], FP16, tag="at", name="at")
                    nc.scalar.activation(at, dts[:, dtl, :], AF.Exp,
                                         scale=acols[:, g:g + 1])
                    gt = gworkp.tile([128, TC + 1], FP16, tag="gt", name="gt")
                    nc.vector.tensor_tensor(
                        gt, xt[:, dtl, t0: t0 + TC + 1], breps[n], AL.mult)
                    dl = gworkp.tile([128, TC], FP16, tag="dl", name="dl")
                    nc.vector.tensor_tensor(
                        dl, gt[:, 1: TC + 1], gt[:, 0: TC], AL.subtract)
                    wt = scanp.tile([128, TC], FP16, tag="wt", name="wt")
                    nc.vector.tensor_tensor_scan(
                        wt, dl, at, wc[:, g:g + 1], op0=AL.add, op1=AL.mult)
                    nc.scalar.copy(wc[:, g:g + 1], wt[:, TC - 1: TC])
                    sct = sctp.tile([128, TC], FP16, tag="sct", name="sct")
                    if itercnt % POOL_SCT == 0:
                        nc.gpsimd.tensor_tensor(sct, wt, creps[n], AL.mult)
                    else:
                        nc.vector.tensor_tensor(sct, wt, creps[n], AL.mult)
                    itercnt += 1
                    for hf in range(NPC):
                        nc.tensor.matmul(
                            pys[hf], dgw[:, g, :],
                            sct[:, hf * PH:(hf + 1) * PH],
                            start=(n == 0), stop=False)

                # D_skip * x
                for hf in range(NPC):
                    nc.tensor.matmul(
                        pys[hf], dskw[:, dtl, :],
                        xt[:, dtl, 1 + t0 + hf * PH: 1 + t0 + (hf + 1) * PH],
                        start=False, stop=False)
                # q correction: y -= x * (qw @ bc)
                qsb = workp.tile([128, TC], FP16, tag="qsb", name="qsb")
                for hf in range(NPC):
                    pq = psB.tile([128, PH], F32, tag="psB")
                    nc.tensor.matmul(pq, qw[:, dtl, :],
                                     bc[:, hf * PH:(hf + 1) * PH],
                                     start=True, stop=True)
                    nc.scalar.copy(qsb[:, hf * PH:(hf + 1) * PH], pq)
                ycr = workp.tile([128, TC], FP16, tag="ycr", name="ycr")
                nc.vector.tensor_tensor(
                    ycr, xt[:, dtl, 1 + t0: 1 + t0 + TC], qsb, AL.mult)
                for hf in range(NPC):
                    nc.tensor.matmul(pys[hf], nident,
                                     ycr[:, hf * PH:(hf + 1) * PH],
                                     start=False, stop=(True))

                yo = youtp.tile([128, TC], FP16, tag="yo", name="yo")
                for hf in range(NPC):
                    nc.scalar.copy(yo[:, hf * PH:(hf + 1) * PH], pys[hf])
                nc.sync.dma_start(
                    y_d[dtl * 128:(dtl + 1) * 128, t0: t0 + TC], yo)


def kernel(x, state, log_A, W_B, W_C, W_dt1, W_dt2, b_dt2, D_skip):
    if "nc" not in _CACHE:
        _CACHE["nc"] = _build_program()
    nc = _CACHE["nc"]

    x = np.asarray(x, np.float32)
    state = np.asarray(state, np.float32)
    A = (-np.exp(np.asarray(log_A, np.float32))).astype(np.float32)
    G = (A + np.float32(1e-8)).astype(np.float32)
    invG = (np.float32(1.0) / G).astype(np.float32)
    W_B = np.asarray(W_B, np.float32)
    W_C = np.asarray(W_C, np.float32)
    W_dt1 = np.asarray(W_dt1, np.float32)
    W_dt2 = np.asarray(W_dt2, np.float32)
    b_dt2 = np.asarray(b_dt2, np.float32)
    D_skip = np.asarray(D_skip, np.float32)

    nident = (-np.eye(128)).astype(np.float16)
    ident16 = np.eye(128).astype(np.float16)
    selbc = np.zeros((128, 2 * N * 128), np.float16)
    for n in range(N):
        selbc[64 + n, n * 128:(n + 1) * 128] = 1.0        # B row n (part 64+n)
        selbc[96 + n, (N + n) * 128:(N + n + 1) * 128] = 1.0  # C row (96+n)

    in_maps = []
    for c in range(NCORES):
        b, h = c // 2, c % 2
        loc = slice(h * DH, (h + 1) * DH)
        perm = np.r_[np.arange(h * DH, (h + 1) * DH),
                     np.arange((1 - h) * DH, (2 - h) * DH)]
        Al = A[loc]                      # [DH, N]
        Gl = G[loc]
        invGl = invG[loc]

        # wall: [W_B.T | W_C.T | W_dt1.T] with permuted rows -> [128, KD*96]
        wallf = np.concatenate(
            [W_dt1.T[perm], W_B.T[perm],
             np.zeros((D, 16), np.float32), W_C.T[perm]], axis=1)  # [D, 112]
        wall = np.ascontiguousarray(
            wallf.reshape(KD, 128, 112).transpose(1, 0, 2).reshape(
                128, KD * 112)).astype(np.float16)

        # w2r: [64, NDT*128]
        w2r = np.ascontiguousarray(
            W_dt2[loc].T.reshape(64, NDT, 128).reshape(64, NDT * 128)
        ).astype(np.float16)

        bd = np.ascontiguousarray(b_dt2[loc].reshape(NDT, 128).T)

        # acols: [128, NDT*N] col (dtl*N+n) = A[dtl*128+p, n]
        acols = np.ascontiguousarray(
            Al.reshape(NDT, 128, N).transpose(1, 0, 2).reshape(128, NDT * N))

        # dgw: diag(invG) per (dtl, n): [128, NDT*N*128]
        dgwm = np.zeros((128, NDT * N, 128), np.float32)
        p = np.arange(128)
        for dtl in range(NDT):
            for n in range(N):
                dgwm[p, dtl * N + n, p] = invGl[dtl * 128 + p, n]
        dgw = np.ascontiguousarray(
            dgwm.reshape(128, NDT * N * 128)).astype(np.float16)

        # dskw: diag(D_skip) per dtl
        dskm = np.zeros((128, NDT, 128), np.float32)
        for dtl in range(NDT):
            dskm[p, dtl, p] = D_skip[loc][dtl * 128 + p]
        dskw = np.ascontiguousarray(
            dskm.reshape(128, NDT * 128)).astype(np.float16)

        # qw: [16, NDT*128]  qw[n, dtl*128+p] = invG[dtl*128+p, n]
        qwm = np.ascontiguousarray(
            invGl.T.reshape(N, NDT, 128).reshape(16, NDT * 128)
        ).astype(np.float16)

        # w0init: G*state0 laid out [128, NDT*N]
        w0 = (Gl * state[b, loc]).reshape(NDT, 128, N).transpose(1, 0, 2)
        w0 = np.ascontiguousarray(w0.reshape(128, NDT * N)).astype(np.float32)

        in_maps.append({
            "x16": np.ascontiguousarray(x[b][:, perm]).astype(np.float16),
            "wall": wall,
            "w2r": w2r,
            "bdt2": bd,
            "acols": acols,
            "dgw": dgw,
            "dskw": dskw,
            "qw": qwm,
            "selbc": selbc,
            "nident": nident,
            "ident16": ident16,
            "w0init": w0,
        })

    res = run_bass_kernel_spmd(nc, in_maps, core_ids=list(range(NCORES)))

    y = np.empty((B, T, D), np.float32)
    for c in range(NCORES):
        b, h = c // 2, c % 2
        y[b][:, h * DH:(h + 1) * DH] = res.results[c]["yT"].T.astype(
            np.float32)
    return y


# revision 13
# speedup vs baseline: 1.7171x; 1.0996x over previous
"""Mamba-1 selective scan on 8 Trainium2 NeuronCores — n-in-free-dim design.

Sharding: core c -> (batch b = c//2, D-half h = c%2): each core owns 512
channels of one batch for the recurrence; projections need the full D=1024.

Math (exact ZOH, rescaled state):
  G = A + 1e-8,  shat := G * s
  a_t = exp(dt_t * A)                           (per d,n,t)
  shat_t = a_t shat_{t-1} + (a_t - 1) ghat_t,   ghat = x * B
  w := shat + ghat  ->  w_t = (delta_t + w_{t-1}) * a_t,
       delta_t = ghat_t - ghat_{t-1}            (hw tensor_tensor_scan)
  y_t[d] = sum_n (1/G)[d,n] (w - ghat) C[n,t] + Dskip[d] x[d,t]
         = [sum_n diag(1/G_n) @ (w_n * crep_n)]  - x*q + Dskip*x
    q[d,t] = sum_n (1/G)[d,n] B[n,t] C[n,t]     (PE matmul of bc = B*C)

On-chip layout: partitions = 128 channels (4 d-tiles per core); n and time
in the free dim: per (n, dtile, chunk) tiles [128, TC].  All elementwise in
fp16 for the DVE 2x mode; scans are 1x; a is fp16 (exp computed f32 on ACT).
xt/pall columns are time-shifted by +1 (col 0 = time -1 = zeros) so the
delta at chunk boundaries needs no carry.
"""

import sys

import numpy as np

sys.path.insert(0, "/opt/trn_rl_repo")

import concourse.bacc as bacc
import concourse.mybir as mybir
import concourse.tile as tile
from concourse.bass_utils import run_bass_kernel_spmd

B, T, D, N, R = 4, 4096, 1024, 16, 64
NCORES = 8
DH = D // 2            # channels per core
NDT = DH // 128        # d-tiles per core (4)
KD = D // 128          # k-tiles over full D for projections (8)
TC = 1024              # time chunk
NCH = T // TC
PH = 512               # psum piece (one bank of f32)
NPC = TC // PH         # psum pieces per chunk (2)
F32 = mybir.dt.float32
FP16 = mybir.dt.float16
AL = mybir.AluOpType
AF = mybir.ActivationFunctionType

# engine assignment for the sct multiply: give Pool every POOL_SCT-th tile
POOL_SCT = 2

_CACHE = {}


def _patch_act_tables():
    """Route Exp+Ln to natural_log_exp_and_others so the softplus (Exp,Ln)
    and the main-loop Exp never force activation-table reloads."""
    import concourse.bacc as _bacc
    from concourse.hw_specs import get_activation_tables as _orig

    def patched(arch):
        t = _orig(arch)
        exp = mybir.ActivationFunctionType.Exp
        ln = mybir.ActivationFunctionType.Ln
        for name, fns in t.items():
            if name != "natural_log_exp_and_others":
                fns.discard(exp)
                fns.discard(ln)
        return t

    _bacc.get_activation_tables = patched


def _build_program():
    _patch_act_tables()
    nc = bacc.Bacc(
        "TRN2",
        target_bir_lowering=False,
        debug=False,
        num_devices=NCORES,
    )

    x_d = nc.dram_tensor("x16", [T, D], FP16, kind="ExternalInput")
    wall_d = nc.dram_tensor("wall", [128, KD * 112], FP16, kind="ExternalInput")
    w2_d = nc.dram_tensor("w2r", [64, NDT * 128], FP16, kind="ExternalInput")
    bd_d = nc.dram_tensor("bdt2", [128, NDT], F32, kind="ExternalInput")
    ac_d = nc.dram_tensor("acols", [128, NDT * N], F32, kind="ExternalInput")
    dgw_d = nc.dram_tensor("dgw", [128, NDT * N * 128], FP16,
                           kind="ExternalInput")
    dsk_d = nc.dram_tensor("dskw", [128, NDT * 128], FP16,
                           kind="ExternalInput")
    qw_d = nc.dram_tensor("qw", [16, NDT * 128], FP16, kind="ExternalInput")
    sel_d = nc.dram_tensor("selbc", [128, 2 * N * 128], FP16, kind="ExternalInput")
    nid_d = nc.dram_tensor("nident", [128, 128], FP16, kind="ExternalInput")
    id16_d = nc.dram_tensor("ident16", [128, 128], FP16, kind="ExternalInput")
    w0_d = nc.dram_tensor("w0init", [128, NDT * N], F32, kind="ExternalInput")
    y_d = nc.dram_tensor("yT", [DH, T], FP16, kind="ExternalOutput")

    with tile.TileContext(nc) as tc:
        _body(tc, x_d, wall_d, w2_d, bd_d, ac_d, dgw_d, dsk_d, qw_d, sel_d,
              nid_d, id16_d, w0_d, y_d)

    nc.compile()
    return nc


def _body(tc, x_d, wall_d, w2_d, bd_d, ac_d, dgw_d, dsk_d, qw_d, sel_d,
          nid_d, id16_d, w0_d, y_d):
    nc = tc.nc

    with (
        tc.tile_pool(name="const", bufs=1) as const,
        tc.tile_pool(name="xload", bufs=2) as xload,
        tc.tile_pool(name="xtmp", bufs=1) as xtmpp,
        tc.tile_pool(name="bcache", bufs=1) as bcache,
        tc.tile_pool(name="dtp", bufs=1) as dtp,
        tc.tile_pool(name="work", bufs=1) as workp,
        tc.tile_pool(name="atp", bufs=2) as atp,
        tc.tile_pool(name="gwork", bufs=2) as gworkp,  # gt+dl
        tc.tile_pool(name="scan", bufs=2) as scanp,
        tc.tile_pool(name="sctp", bufs=2) as sctp,
        tc.tile_pool(name="yout", bufs=1) as youtp,
        tc.tile_pool(name="psA", bufs=1, space="PSUM") as psA,
        tc.tile_pool(name="psB", bufs=2, space="PSUM") as psB,
        tc.tile_pool(name="psY", bufs=2, space="PSUM") as psY,
    ):
        # ---- constants ----
        wall = const.tile([128, KD, 112], FP16)
        nc.gpsimd.dma_start(wall, wall_d.ap().rearrange("p (k m) -> p k m",
                                                        k=KD))
        w2r = const.tile([64, NDT, 128], FP16)
        nc.gpsimd.dma_start(w2r, w2_d.ap().rearrange("p (d m) -> p d m",
                                                     d=NDT))
        bdt2 = const.tile([128, NDT], F32)
        nc.gpsimd.dma_start(bdt2, bd_d[:, :])
        acols = const.tile([128, NDT * N], F32)
        nc.gpsimd.dma_start(acols, ac_d[:, :])
        dgw = const.tile([128, NDT * N, 128], FP16)
        nc.gpsimd.dma_start(dgw, dgw_d.ap().rearrange("p (g m) -> p g m",
                                                      g=NDT * N))
        dskw = const.tile([128, NDT, 128], FP16)
        nc.gpsimd.dma_start(dskw, dsk_d.ap().rearrange("p (d m) -> p d m",
                                                       d=NDT))
        qw = const.tile([16, NDT, 128], FP16)
        nc.gpsimd.dma_start(qw, qw_d.ap().rearrange("p (d m) -> p d m",
                                                    d=NDT))
        selbc = const.tile([128, 2 * N, 128], FP16)
        nc.gpsimd.dma_start(selbc, sel_d.ap().rearrange("p (n m) -> p n m",
                                                        n=2 * N))
        nident = const.tile([128, 128], FP16)
        nc.gpsimd.dma_start(nident, nid_d[:, :])
        ident16 = const.tile([128, 128], FP16)
        nc.gpsimd.dma_start(ident16, id16_d[:, :])
        wc = const.tile([128, NDT * N], F32)
        nc.gpsimd.dma_start(wc, w0_d[:, :])

        # ---- transposes + projections (time-shifted by +1 col) ----
        # xt[:, dtl, 1+t] = x[t, dtl*128 + p]; col 0 = 0
        xt = const.tile([128, NDT, T + 1], FP16)
        nc.vector.memset(xt[:, :, 0:1], 0.0)
        # pall[0:64]=xr, [64:80]=Bt, [96:112]=Ct (80:96 pad); col 0 = 0
        pall = const.tile([112, T + 1], FP16)
        nc.vector.memset(pall[:, 0:1], 0.0)

        for tp in range(T // PH):
            xls = []
            for j in range(4):
                xld = xload.tile([128, D], FP16, tag=f"xld{j}",
                                 name=f"xld{j}")
                nc.sync.dma_start(
                    xld, x_d[tp * PH + j * 128: tp * PH + (j + 1) * 128, :])
                xls.append(xld)
            ktiles = []
            for k in range(KD):
                ptr = psB.tile([128, PH], FP16, tag="psB")
                for j in range(4):
                    nc.tensor.transpose(
                        ptr[:, j * 128:(j + 1) * 128],
                        xls[j][:, k * 128:(k + 1) * 128], ident16)
                if k < NDT:
                    dst = xt[:, k, 1 + tp * PH: 1 + (tp + 1) * PH]
                    nc.scalar.copy(dst, ptr)
                    ktiles.append(xt[:, k, 1 + tp * PH: 1 + (tp + 1) * PH])
                else:
                    xtm = xtmpp.tile([128, PH], FP16, tag=f"xtm{k}",
                                     name=f"xtm{k}")
                    nc.scalar.copy(xtm, ptr)
                    ktiles.append(xtm)
            pp = psB.tile([112, PH], F32, tag="psB")
            for k in range(KD):
                nc.tensor.matmul(pp, wall[:, k, :], ktiles[k],
                                 start=(k == 0), stop=(k == KD - 1))
            nc.scalar.copy(pall[:, 1 + tp * PH: 1 + (tp + 1) * PH], pp)

        itercnt = 0
        for ch in range(NCH):
            t0 = ch * TC

            # ---- dt for this chunk: softplus(w2 @ xr + b) ----
            dts = dtp.tile([128, NDT, TC], FP16, tag="dts", name="dts")
            for dtl in range(NDT):
                for hf in range(NPC):
                    sl = slice(1 + t0 + hf * PH, 1 + t0 + (hf + 1) * PH)
                    pdt = psB.tile([128, PH], F32, tag="psB")
                    nc.tensor.matmul(pdt, w2r[:, dtl, :], pall[0:64, sl],
                                     start=True, stop=True)
                    dsl = dts[:, dtl, hf * PH:(hf + 1) * PH]
                    nc.scalar.activation(dsl, pdt, AF.Exp,
                                         bias=bdt2[:, dtl:dtl + 1], scale=1.0)
                nc.scalar.activation(dts[:, dtl, :], dts[:, dtl, :],
                                     AF.Ln, bias=1.0, scale=1.0)

            # ---- bc = B*C for the q correction (copies realign base) ----
            btc = workp.tile([16, TC], FP16, tag="btc", name="btc")
            nc.scalar.copy(btc, pall[64:80, 1 + t0: 1 + t0 + TC])
            ctc = workp.tile([16, TC], FP16, tag="ctc", name="ctc")
            nc.scalar.copy(ctc, pall[96:112, 1 + t0: 1 + t0 + TC])
            bc = workp.tile([16, TC], FP16, tag="bc", name="bc")
            nc.vector.tensor_tensor(bc, btc, ctc, AL.mult)

            # ---- broadcast B and C rows to 128 partitions ----
            # brep1[n] covers times t0-1 .. t0+TC-1  (pall cols t0..t0+TC)
            # crep[n]  covers times t0   .. t0+TC-1  (pall cols 1+t0..)
            breps, creps = [], []
            for n in range(N):
                br = bcache.tile([128, TC + 1], FP16, tag=f"br{n}",
                                 name=f"br{n}")
                pb = psA.tile([128, TC], F32, tag="psA")
                for hf in range(NPC):
                    nc.tensor.matmul(
                        pb[:, hf * PH:(hf + 1) * PH], selbc[64:112, n, :],
                        pall[64:112, t0 + hf * PH: t0 + (hf + 1) * PH],
                        start=True, stop=True, tile_position=(64, 0))
                nc.scalar.copy(br[:, 0:TC], pb)
                pbl = psB.tile([128, PH], F32, tag="psB")
                nc.tensor.matmul(pbl[:, 0:1], selbc[64:112, n, :],
                                 pall[64:112, t0 + TC: t0 + TC + 1],
                                 start=True, stop=True, tile_position=(64, 0))
                nc.scalar.copy(br[:, TC: TC + 1], pbl[:, 0:1])
                breps.append(br)

                cr = bcache.tile([128, TC], FP16, tag=f"cr{n}",
                                 name=f"cr{n}")
                pc = psA.tile([128, TC], F32, tag="psA")
                for hf in range(NPC):
                    nc.tensor.matmul(
                        pc[:, hf * PH:(hf + 1) * PH], selbc[64:112, N + n, :],
                        pall[64:112, 1 + t0 + hf * PH: 1 + t0 + (hf + 1) * PH],
                        start=True, stop=True, tile_position=(64, 0))
                nc.scalar.copy(cr, pc)
                creps.append(cr)

            # ---- recurrence per (dtile, n) ----
            for dtl in range(NDT):
                py = psY.tile([128, TC], F32, tag="psY", name="py")
                pys = [py[:, hf * PH:(hf + 1) * PH] for hf in range(NPC)]
                for n in range(N):
                    g = dtl * N + n
                    at = atp.tile([128, TC], F32, tag="at", name="at")
                    nc.scalar.activation(at, dts[:, dtl, :], AF.Exp,
                                         scale=acols[:, g:g + 1])
                    gt = gworkp.tile([128, TC + 1], FP16, tag="gt", name="gt")
                    nc.vector.tensor_tensor(
                        gt, xt[:, dtl, t0: t0 + TC + 1], breps[n], AL.mult)
                    dl = gworkp.tile([128, TC], FP16, tag="dl", name="dl")
                    nc.vector.tensor_tensor(
                        dl, gt[:, 1: TC + 1], gt[:, 0: TC], AL.subtract)
                    wt = scanp.tile([128, TC], FP16, tag="wt", name="wt")
                    nc.vector.tensor_tensor_scan(
                        wt, dl, at, wc[:, g:g + 1], op0=AL.add, op1=AL.mult)
                    nc.scalar.copy(wc[:, g:g + 1], wt[:, TC - 1: TC])
                    sct = sctp.tile([128, TC], FP16, tag="sct", name="sct")
                    nc.gpsimd.tensor_tensor(sct, wt, creps[n], AL.mult)
                    itercnt += 1
                    for hf in range(NPC):
                        nc.tensor.matmul(
                            pys[hf], dgw[:, g, :],
                            sct[:, hf * PH:(hf + 1) * PH],
                            start=(n == 0), stop=False)

                # D_skip * x
                for hf in range(NPC):
                    nc.tensor.matmul(
                        pys[hf], dskw[:, dtl, :],
                        xt[:, dtl, 1 + t0 + hf * PH: 1 + t0 + (hf + 1) * PH],
                        start=False, stop=False)
                # q correction: y -= x * (qw @ bc)
                qsb = workp.tile([128, TC], FP16, tag="qsb", name="qsb")
                for hf in range(NPC):
                    pq = psB.tile([128, PH], F32, tag="psB")
                    nc.tensor.matmul(pq, qw[:, dtl, :],
                                     bc[:, hf * PH:(hf + 1) * PH],
                                     start=True, stop=True)
                    nc.scalar.copy(qsb[:, hf * PH:(hf + 1) * PH], pq)
                ycr = workp.tile([128, TC], FP16, tag="ycr", name="ycr")
                nc.vector.tensor_tensor(
                    ycr, xt[:, dtl, 1 + t0: 1 + t0 + TC], qsb, AL.mult)
                for hf in range(NPC):
                    nc.tensor.matmul(pys[hf], nident,
                                     ycr[:, hf * PH:(hf + 1) * PH],
                                     start=False, stop=(True))

                yo = youtp.tile([128, TC], FP16, tag="yo", name="yo")
                nc.scalar.copy(yo, py)
                nc.sync.dma_start(
                    y_d[dtl * 128:(dtl + 1) * 128, t0: t0 + TC], yo)


def kernel(x, state, log_A, W_B, W_C, W_dt1, W_dt2, b_dt2, D_skip):
    if "nc" not in _CACHE:
        _CACHE["nc"] = _build_program()
    nc = _CACHE["nc"]

    x = np.asarray(x, np.float32)
    state = np.asarray(state, np.float32)
    A = (-np.exp(np.asarray(log_A, np.float32))).astype(np.float32)
    G = (A + np.float32(1e-8)).astype(np.float32)
    invG = (np.float32(1.0) / G).astype(np.float32)
    W_B = np.asarray(W_B, np.float32)
    W_C = np.asarray(W_C, np.float32)
    W_dt1 = np.asarray(W_dt1, np.float32)
    W_dt2 = np.asarray(W_dt2, np.float32)
    b_dt2 = np.asarray(b_dt2, np.float32)
    D_skip = np.asarray(D_skip, np.float32)

    nident = (-np.eye(128)).astype(np.float16)
    ident16 = np.eye(128).astype(np.float16)
    selbc = np.zeros((128, 2 * N * 128), np.float16)
    for n in range(N):
        selbc[64 + n, n * 128:(n + 1) * 128] = 1.0        # B row n (part 64+n)
        selbc[96 + n, (N + n) * 128:(N + n + 1) * 128] = 1.0  # C row (96+n)

    in_maps = []
    for c in range(NCORES):
        b, h = c // 2, c % 2
        loc = slice(h * DH, (h + 1) * DH)
        perm = np.r_[np.arange(h * DH, (h + 1) * DH),
                     np.arange((1 - h) * DH, (2 - h) * DH)]
        Al = A[loc]                      # [DH, N]
        Gl = G[loc]
        invGl = invG[loc]

        # wall: [W_B.T | W_C.T | W_dt1.T] with permuted rows -> [128, KD*96]
        wallf = np.concatenate(
            [W_dt1.T[perm], W_B.T[perm],
             np.zeros((D, 16), np.float32), W_C.T[perm]], axis=1)  # [D, 112]
        wall = np.ascontiguousarray(
            wallf.reshape(KD, 128, 112).transpose(1, 0, 2).reshape(
                128, KD * 112)).astype(np.float16)

        # w2r: [64, NDT*128]
        w2r = np.ascontiguousarray(
            W_dt2[loc].T.reshape(64, NDT, 128).reshape(64, NDT * 128)
        ).astype(np.float16)

        bd = np.ascontiguousarray(b_dt2[loc].reshape(NDT, 128).T)

        # acols: [128, NDT*N] col (dtl*N+n) = A[dtl*128+p, n]
        acols = np.ascontiguousarray(
            Al.reshape(NDT, 128, N).transpose(1, 0, 2).reshape(128, NDT * N))

        # dgw: diag(invG) per (dtl, n): [128, NDT*N*128]
        dgwm = np.zeros((128, NDT * N, 128), np.float32)
        p = np.arange(128)
        for dtl in range(NDT):
            for n in range(N):
                dgwm[p, dtl * N + n, p] = invGl[dtl * 128 + p, n]
        dgw = np.ascontiguousarray(
            dgwm.reshape(128, NDT * N * 128)).astype(np.float16)

        # dskw: diag(D_skip) per dtl
        dskm = np.zeros((128, NDT, 128), np.float32)
        for dtl in range(NDT):
            dskm[p, dtl, p] = D_skip[loc][dtl * 128 + p]
        dskw = np.ascontiguousarray(
            dskm.reshape(128, NDT * 128)).astype(np.float16)

        # qw: [16, NDT*128]  qw[n, dtl*128+p] = invG[dtl*128+p, n]
        qwm = np.ascontiguousarray(
            invGl.T.reshape(N, NDT, 128).reshape(16, NDT * 128)
        ).astype(np.float16)

        # w0init: G*state0 laid out [128, NDT*N]
        w0 = (Gl * state[b, loc]).reshape(NDT, 128, N).transpose(1, 0, 2)
        w0 = np.ascontiguousarray(w0.reshape(128, NDT * N)).astype(np.float32)

        in_maps.append({
            "x16": np.ascontiguousarray(x[b][:, perm]).astype(np.float16),
            "wall": wall,
            "w2r": w2r,
            "bdt2": bd,
            "acols": acols,
            "dgw": dgw,
            "dskw": dskw,
            "qw": qwm,
            "selbc": selbc,
            "nident": nident,
            "ident16": ident16,
            "w0init": w0,
        })

    res = run_bass_kernel_spmd(nc, in_maps, core_ids=list(range(NCORES)))

    y = np.empty((B, T, D), np.float32)
    for c in range(NCORES):
        b, h = c // 2, c % 2
        y[b][:, h * DH:(h + 1) * DH] = res.results[c]["yT"].T.astype(
            np.float32)
    return y
